# revision 30
# baseline (speedup 1.0000x reference)
"""Trainium2 Bass kernel for nn_Metamorph_parameterReinforcer.

Math notes (exact identities, verified against the reference to ~8e-6):
  - The reference's einsum 'bfp,mn->bfm' contracts p and n INDEPENDENTLY:
    y[b,f,0] = (sum_p fft(data)[b,f,p]) * (sum_n w_fft[0,n]).
  - sum_k FFT_forward(x)[k] == x[0] exactly, so the whole FFT stage collapses
    to an elementwise complex-tanh chain on v = model_p[:, 0]:
      x_{i+1} = Re(tanh(x_i * s_i)),  s_i = sum(w_fft_i)
    with Re(tanh(a+ib)) = tanh(2a) / (1 + cos(2b) * sech(2a)),
    sech(2a) = sqrt(1 - tanh(2a)^2)  (saturation-safe).
  - Then h = tanh(lin1_w @ x3 + lin1_b); y = lin2_w @ h + lin2_b.

Sharding (8 cores, zero device-to-device communication):
  - chain stage: replicated (tiny); the w_fft sums are computed on device.
  - lin1 row-sharded: core c computes h[c*512:(c+1)*512].
  - lin2 CONTRACTION-sharded: core c holds lin2_w[:, c*512:(c+1)*512], which
    contracts exactly against the h shard core c just produced. Each core
    emits a partial y (16384,); the host sums the 8 partials (lin2_b is
    added on device by core 0 only, via a per-core bias input).
  Per-core HBM traffic = 4.2MB (lin1 shard) + 33.5MB (lin2 shard) — the
  minimum possible; this is the memory-roofline distribution.
"""

import json
import math
import numpy as np

import concourse.bass as bass
import concourse.mybir as mybir
import concourse.tile as tile

N_CORES = 8
NO_LAYERS = 2048
MODES = 4096
APL = 8
M_OUT = NO_LAYERS * APL          # 16384
SH = MODES // N_CORES            # 512  (lin1 rows / lin2 contraction per core)
F32 = mybir.dt.float32

W2_NTILE = 2048                  # lin2 free-dim tile (1MB DMA transfers)
W2_NJ = M_OUT // W2_NTILE        # 8 column stripes
W2_NB = W2_NTILE // 512          # 4 psum blocks per stripe


def _legalize_wait_counts(raw: bytes, max_w: int = 1) -> bytes:
    """This walrus build's codegen accepts only one embedded sem wait per
    instruction ("Too many sync wait commands"). Split any instruction with
    more waits by hoisting the extras onto preceding single-wait NoOps on
    the same engine — semantically identical (all waits must pass before the
    instruction runs; engine queues are in-order)."""
    m = json.loads(raw)
    ctr = 0
    for fn in m["functions"]:
        for bb in fn["blocks"]:
            out = []
            for inst in bb["instructions"]:
                si = inst.get("sync_info")
                if si and si.get("on_wait") and len(si["on_wait"]) > max_w:
                    waits = si["on_wait"]
                    for w in waits[:-max_w]:
                        ctr += 1
                        out.append(
                            {
                                "name": f"I-legwait-{ctr}",
                                "opcode": "NoOp",
                                "engine": inst["engine"],
                                "ins": [],
                                "outs": [],
                                "sync_info": {"on_wait": [w], "on_update": []},
                            }
                        )
                    si["on_wait"] = waits[-max_w:]
                out.append(inst)
            bb["instructions"] = out
    return json.dumps(m).encode()


def build_nc():
    """Build the single-core Bass program (same program on all 8 cores)."""
    nc = bass.Bass()

    xin = nc.declare_dram_parameter("xin", [128, 16], F32, isOutput=False)
    wft = nc.declare_dram_parameter("wft", [128, 192], F32, isOutput=False)
    l1wt = nc.declare_dram_parameter("l1wt", [128, 16 * SH], F32, isOutput=False)
    l1b = nc.declare_dram_parameter("l1b", [128, 4], F32, isOutput=False)
    w2t = nc.declare_dram_parameter("w2t", [128, 4 * M_OUT], F32, isOutput=False)
    l2b = nc.declare_dram_parameter("l2b", [1, M_OUT], F32, isOutput=False)
    yout = nc.declare_dram_parameter("yout", [1, M_OUT], F32, isOutput=True)

    AX = mybir.AxisListType
    OP = mybir.AluOpType
    AF = mybir.ActivationFunctionType

    with tile.TileContext(nc) as tc:
        with (
            tc.tile_pool(name="singles", bufs=1) as singles,
            tc.tile_pool(name="chain", bufs=3) as chain,
            tc.tile_pool(name="w2pool", bufs=13) as w2pool,
            tc.tile_pool(name="biasp", bufs=1) as biasp,
            tc.tile_pool(name="ps1", bufs=1, space="PSUM") as ps1,
            tc.tile_pool(name="psy", bufs=5, space="PSUM") as psy,
        ):
            # ---- small input loads -------------------------------------
            xin_sb = singles.tile([128, 16], F32)
            nc.sync.dma_start(out=xin_sb, in_=xin[:, :])
            wft_sb = singles.tile([128, 192], F32)
            nc.sync.dma_start(out=wft_sb, in_=wft[:, :])
            l1b_sb = singles.tile([128, 4], F32)
            nc.sync.dma_start(out=l1b_sb, in_=l1b[:, :])

            ones = singles.tile([128, 128], F32)
            nc.vector.memset(ones, 1.0)
            pihalf = singles.tile([128, 1], F32)
            nc.vector.memset(pihalf, math.pi / 2)

            # ---- w_fft sums + broadcast to all partitions --------------
            # wft_sb[p, j*6+c] = wfT[j*128+p, c]; reduce over j then over
            # partitions (via ones-matmul, which also broadcasts), giving
            # bc2[m, c] = 2 * sum_k wfT[k, c] on every partition m.
            part6 = singles.tile([128, 6], F32)
            nc.vector.tensor_reduce(
                out=part6,
                in_=wft_sb.rearrange("p (j c) -> p c j", c=6),
                axis=AX.X,
                op=OP.add,
            )
            psum_bc = ps1.tile([128, 6], F32)
            nc.tensor.matmul(psum_bc, lhsT=ones, rhs=part6, start=True, stop=True)
            # bc2 on DVE (not ACT): the ACT instruction struct supports only
            # a single sem wait, so every ACT op must have all its
            # cross-engine producers on one proc (DVE).
            bc2 = singles.tile([128, 6], F32)
            nc.vector.tensor_scalar_mul(bc2, psum_bc, 2.0)

            # ---- 3-layer complex-tanh chain on [128, 16] ---------------
            x0 = chain.tile([128, 16], F32, tag="x0")
            nc.vector.tensor_copy(x0, xin_sb)
            x = x0
            for i in range(3):
                sr2 = bc2[:, 2 * i : 2 * i + 1]
                si2 = bc2[:, 2 * i + 1 : 2 * i + 2]
                T = chain.tile([128, 16], F32, tag="T")
                nc.scalar.activation(T, x, AF.Tanh, scale=sr2)
                b2 = chain.tile([128, 16], F32, tag="b2")
                nc.vector.tensor_scalar_mul(b2, x, si2)
                # cos(z): clamp z to ±30 (beyond that sech(2a) < 1e-23 kills
                # the cos term since |a| >= 0.93|z|), then start from
                # cos(z/32) = sin(z/32 + π/2) — argument within the Sin
                # table's [-π, π] domain — and apply the double-angle formula
                # cos(2θ) = 2cos²θ − 1 five times.
                b2c = chain.tile([128, 16], F32, tag="b2c")
                nc.vector.tensor_scalar(b2c, b2, -30.0, 30.0, OP.max, OP.min)
                C = chain.tile([128, 16], F32, tag="C")
                nc.scalar.activation(C, b2c, AF.Sin, bias=pihalf, scale=1.0 / 32.0)
                for _ in range(5):
                    Csq = chain.tile([128, 16], F32, tag="Csq")
                    nc.vector.tensor_mul(Csq, C, C)
                    C = chain.tile([128, 16], F32, tag="C")
                    nc.vector.tensor_scalar(C, Csq, 2.0, 1.0, OP.mult, OP.subtract)
                T2 = chain.tile([128, 16], F32, tag="T2")
                nc.vector.tensor_mul(T2, T, T)
                S = chain.tile([128, 16], F32, tag="S")
                nc.scalar.activation(S, T2, AF.Sqrt, bias=1.0, scale=-1.0)
                CS = chain.tile([128, 16], F32, tag="CS")
                nc.vector.tensor_mul(CS, C, S)
                D = chain.tile([128, 16], F32, tag="D")
                nc.vector.tensor_scalar_add(D, CS, 1.0)
                Rv = chain.tile([128, 16], F32, tag="Rv")
                nc.vector.reciprocal(Rv, D)
                xn = chain.tile([128, 16], F32, tag="xn")
                nc.vector.tensor_mul(xn, T, Rv)
                x = xn

            # The PE matmul LoadWeights struct also supports only one sem
            # wait. Real matmuls depend on both a weight-tile DMA and a
            # DVE/ACT-produced operand; that would be two waits. Fix: a
            # "warmup" matmul per weight tile whose only cross-engine dep is
            # that tile's DMA — after it, PE has observed the DMA lane tick
            # and the real matmuls need at most one wait.
            psum_dummy = ps1.tile([128, 1], F32)

            def warm(t):
                nc.tensor.matmul(
                    psum_dummy,
                    lhsT=t[:, 0:128],
                    rhs=ones[:, 0:1],
                    start=True,
                    stop=True,
                )

            # Same single-wait story on DVE: a tensor_tensor may depend on
            # PE (psum) + a DMA'd operand + a recycled-slot WAR — too many.
            # A 1-element DVE "touch" read of a freshly DMA'd tile makes DVE
            # observe that DMA lane's tick so the real op needs only the PE
            # wait.
            def touch(ap):
                tt = chain.tile([1, 1], F32, tag="touch")
                nc.vector.tensor_copy(tt, ap[0:1, 0:1])

            touch(l1b_sb)

            # lin1 weights stream through the same pool as lin2 tiles
            l1w_tiles = []
            for i in range(4):
                t = w2pool.tile([128, W2_NTILE], F32, tag="wtile")
                nc.sync.dma_start(
                    out=t, in_=l1wt[:, i * W2_NTILE : (i + 1) * W2_NTILE]
                )
                warm(t)
                l1w_tiles.append(t)

            # ---- lin1 shard: h[m,n] = tanh(sum_k l1wT[k, n*128+m]*x3[k] + b)
            psum_h = ps1.tile([128, 4], F32)
            for n in range(4):
                for kc in range(16):
                    lt = l1w_tiles[kc // 4]
                    base = (kc % 4) * SH
                    nc.tensor.matmul(
                        psum_h[:, n : n + 1],
                        lhsT=lt[:, base + n * 128 : base + (n + 1) * 128],
                        rhs=x[:, kc : kc + 1],
                        start=(kc == 0),
                        stop=(kc == 15),
                    )
            hb = singles.tile([128, 4], F32)
            nc.vector.tensor_add(hb, psum_h, l1b_sb)
            h = singles.tile([128, 4], F32)
            nc.scalar.activation(h, hb, AF.Tanh)

            # ---- lin2 shard: y_partial[n] = sum_k W2cT[k, n] * h[k] + bias
            # Output accumulates into a single never-recycled SBUF slab (no
            # WAR waits); the bias is added on the PE as an extra K=1 matmul
            # in each accumulation group.
            yslab = singles.tile([1, M_OUT], F32)
            for j in range(W2_NJ):
                wt = []
                for kc in range(4):
                    t = w2pool.tile([128, W2_NTILE], F32, tag="wtile")
                    nc.sync.dma_start(
                        out=t,
                        in_=w2t[:, kc * M_OUT + j * W2_NTILE : kc * M_OUT + (j + 1) * W2_NTILE],
                    )
                    warm(t)
                    wt.append(t)
                bias_t = biasp.tile([1, W2_NTILE], F32, tag="bias")
                nc.sync.dma_start(
                    out=bias_t, in_=l2b[0:1, j * W2_NTILE : (j + 1) * W2_NTILE]
                )
                # PE-observe the bias DMA so the bias matmuls need no wait
                nc.tensor.matmul(
                    psum_dummy[0:1, 0:1],
                    lhsT=bias_t[0:1, 0:1],
                    rhs=ones[0:1, 0:1],
                    start=True,
                    stop=True,
                )
                for nb in range(W2_NB):
                    ps = psy.tile([1, 512], F32, tag="psy")
                    for kc in range(4):
                        nc.tensor.matmul(
                            ps,
                            lhsT=h[:, kc : kc + 1],
                            rhs=wt[kc][:, nb * 512 : (nb + 1) * 512],
                            start=(kc == 0),
                            stop=False,
                        )
                    nc.tensor.matmul(
                        ps,
                        lhsT=ones[0:1, 0:1],
                        rhs=bias_t[0:1, nb * 512 : (nb + 1) * 512],
                        start=False,
                        stop=True,
                    )
                    n0 = j * W2_NTILE + nb * 512
                    nc.vector.tensor_copy(yslab[0:1, n0 : n0 + 512], ps)
            nc.sync.dma_start(out=yout[0:1, :], in_=yslab)

    fixed = _legalize_wait_counts(nc.to_json_bytes())
    nc.to_json_bytes = lambda: fixed
    return nc


def make_in_maps(inputs):
    """Host-side shard/relayout of the full inputs into per-core arrays."""
    model_p = np.asarray(inputs["model_p"])
    v = np.ascontiguousarray(model_p[:, 0]).astype(np.float32)          # (2048,)
    xin = np.ascontiguousarray(v.reshape(16, 128).T)                    # (128,16)

    wfs = []
    for k in ("w_fft_0", "w_fft_1", "w_fft_2"):
        w = np.asarray(inputs[k]).reshape(MODES)
        wfs.append(np.ascontiguousarray(w.real).astype(np.float32))
        wfs.append(np.ascontiguousarray(w.imag).astype(np.float32))
    wfT = np.stack(wfs, axis=1)                                         # (4096, 6)
    wft = np.ascontiguousarray(
        wfT.reshape(32, 128, 6).transpose(1, 0, 2).reshape(128, 192)
    )

    lin1_w = np.asarray(inputs["lin1_w"], dtype=np.float32)             # (4096, 2048)
    lin1_b = np.asarray(inputs["lin1_b"], dtype=np.float32)             # (4096,)
    lin2_w = np.asarray(inputs["lin2_w"], dtype=np.float32)             # (16384, 4096)
    lin2_b = np.asarray(inputs["lin2_b"], dtype=np.float32)             # (16384,)

    in_maps = []
    for c in range(N_CORES):
        l1wT = lin1_w[c * SH : (c + 1) * SH, :].T                       # (2048, 512)
        l1wt = np.ascontiguousarray(
            l1wT.reshape(16, 128, SH).transpose(1, 0, 2).reshape(128, 16 * SH)
        )
        l1bc = np.ascontiguousarray(
            lin1_b[c * SH : (c + 1) * SH].reshape(4, 128).T
        )                                                               # (128, 4)
        w2T = lin2_w[:, c * SH : (c + 1) * SH].T                        # (512, 16384)
        w2tc = np.ascontiguousarray(
            w2T.reshape(4, 128, M_OUT).transpose(1, 0, 2).reshape(128, 4 * M_OUT)
        )
        if c == 0:
            l2bc = np.ascontiguousarray(lin2_b.reshape(1, M_OUT))
        else:
            l2bc = np.zeros((1, M_OUT), np.float32)
        in_maps.append(
            {
                "xin": xin,
                "wft": wft,
                "l1wt": l1wt,
                "l1b": l1bc,
                "w2t": w2tc,
                "l2b": l2bc,
            }
        )
    return in_maps


_CACHE = {}


def kernel(**inputs) -> np.ndarray:
    from concourse.bass_utils import run_bass_kernel_spmd

    if "nc" not in _CACHE:
        _CACHE["nc"] = build_nc()
    nc = _CACHE["nc"]

    in_maps = make_in_maps(inputs)
    res = run_bass_kernel_spmd(nc, in_maps, list(range(N_CORES)))
    y = np.zeros(M_OUT, np.float64)
    for c in range(N_CORES):
        y += res.results[c]["yout"].reshape(M_OUT).astype(np.float64)
    return y.astype(np.float32).reshape(1, NO_LAYERS, APL)


# revision 56
# speedup vs baseline: 38.7688x; 38.7688x over previous
"""Trainium2 Bass kernel for nn_Metamorph_parameterReinforcer.

Math notes (exact identities, verified against the reference to ~8e-6):
  - The reference's einsum 'bfp,mn->bfm' contracts p and n INDEPENDENTLY:
    y[b,f,0] = (sum_p fft(data)[b,f,p]) * (sum_n w_fft[0,n]).
  - sum_k FFT_forward(x)[k] == x[0] exactly, so the whole FFT stage collapses
    to an elementwise complex-tanh chain on v = model_p[:, 0]:
      x_{i+1} = Re(tanh(x_i * s_i)),  s_i = sum(w_fft_i)
    with Re(tanh(a+ib)) = tanh(2a) / (1 + cos(2b) * sech(2a)),
    sech(2a) = sqrt(1 - tanh(2a)^2)  (saturation-safe).
  - Then h = tanh(lin1_w @ x3 + lin1_b); y = lin2_w @ h + lin2_b.

Sharding (8 cores, zero device-to-device communication):
  - chain stage: replicated (tiny); the w_fft sums are computed on device.
  - lin1 row-sharded: core c computes h[c*512:(c+1)*512].
  - lin2 CONTRACTION-sharded: core c holds lin2_w[:, c*512:(c+1)*512], which
    contracts exactly against the h shard core c just produced. Each core
    emits a partial y (16384,); the host sums the 8 partials (lin2_b is
    added on device by core 0 only, via a per-core bias input).
  Per-core HBM traffic = 4.2MB (lin1 shard) + 33.5MB (lin2 shard) — the
  minimum possible; this is the memory-roofline distribution.
"""

import json
import math
import numpy as np

import concourse.bass as bass
import concourse.mybir as mybir
import concourse.tile as tile

N_CORES = 8
NO_LAYERS = 2048
MODES = 4096
APL = 8
M_OUT = NO_LAYERS * APL          # 16384
SH = MODES // N_CORES            # 512  (lin1 rows / lin2 contraction per core)
F32 = mybir.dt.float32

W2_NTILE = 2048                  # lin2 free-dim tile (1MB DMA transfers)
W2_NB = W2_NTILE // 512          # 4 psum blocks per stripe
PE_NJ = 4                        # stripes on the PE -> outputs [0, 8192)
NPE = PE_NJ * W2_NTILE           # 6144
NRB = (M_OUT - NPE) // 128       # 80 row-blocks handled on the DVE


def _legalize_wait_counts(raw: bytes, max_w: int = 1) -> bytes:
    """This walrus build's codegen accepts only one embedded sem wait per
    instruction ("Too many sync wait commands"). Split any instruction with
    more waits by hoisting the extras onto preceding single-wait NoOps on
    the same engine — semantically identical (all waits must pass before the
    instruction runs; engine queues are in-order)."""
    m = json.loads(raw)
    ctr = 0
    for fn in m["functions"]:
        for bb in fn["blocks"]:
            out = []
            for inst in bb["instructions"]:
                si = inst.get("sync_info")
                if si and si.get("on_wait") and len(si["on_wait"]) > max_w:
                    waits = si["on_wait"]
                    for w in waits[:-max_w]:
                        ctr += 1
                        out.append(
                            {
                                "name": f"I-legwait-{ctr}",
                                "opcode": "NoOp",
                                "engine": inst["engine"],
                                "ins": [],
                                "outs": [],
                                "sync_info": {"on_wait": [w], "on_update": []},
                            }
                        )
                    si["on_wait"] = waits[-max_w:]
                out.append(inst)
            bb["instructions"] = out
    return json.dumps(m).encode()


def build_nc():
    """Build the single-core Bass program (same program on all 8 cores)."""
    nc = bass.Bass()

    xin = nc.declare_dram_parameter("xin", [128, 16], F32, isOutput=False)
    wft = nc.declare_dram_parameter("wft", [128, 192], F32, isOutput=False)
    l1wt = nc.declare_dram_parameter("l1wt", [128, 16 * SH], F32, isOutput=False)
    l1b = nc.declare_dram_parameter("l1b", [128, 4], F32, isOutput=False)
    w2t = nc.declare_dram_parameter("w2t", [128, 4 * NPE], F32, isOutput=False)
    w2v = nc.declare_dram_parameter("w2v", [128, NRB * SH], F32, isOutput=False)
    l2b = nc.declare_dram_parameter("l2b", [1, M_OUT], F32, isOutput=False)
    b2v = nc.declare_dram_parameter("b2v", [128, NRB], F32, isOutput=False)
    yout = nc.declare_dram_parameter("yout", [1, NPE], F32, isOutput=True)
    yout2 = nc.declare_dram_parameter("yout2", [128, NRB], F32, isOutput=True)
    h_dram = nc.dram_tensor("h_scratch", [1, SH], F32)

    AX = mybir.AxisListType
    OP = mybir.AluOpType
    AF = mybir.ActivationFunctionType

    with tile.TileContext(nc) as tc:
        with (
            tc.tile_pool(name="singles", bufs=1) as singles,
            tc.tile_pool(name="chain", bufs=3) as chain,
            tc.tile_pool(name="w2pool", bufs=9) as w2pool,
            tc.tile_pool(name="w2vp", bufs=4) as w2vp,
            tc.tile_pool(name="biasp", bufs=1) as biasp,
            tc.tile_pool(name="ps1", bufs=1, space="PSUM") as ps1,
            tc.tile_pool(name="psy", bufs=5, space="PSUM") as psy,
        ):
            # ---- small input loads -------------------------------------
            xin_sb = singles.tile([128, 16], F32)
            nc.sync.dma_start(out=xin_sb, in_=xin[:, :])
            wft_sb = singles.tile([128, 192], F32)
            nc.sync.dma_start(out=wft_sb, in_=wft[:, :])
            l1b_sb = singles.tile([128, 4], F32)
            nc.sync.dma_start(out=l1b_sb, in_=l1b[:, :])
            b2v_sb = singles.tile([128, NRB], F32)
            nc.sync.dma_start(out=b2v_sb, in_=b2v[:, :])

            ones = singles.tile([128, 128], F32)
            nc.vector.memset(ones, 1.0)
            pihalf = singles.tile([128, 1], F32)
            nc.vector.memset(pihalf, math.pi / 2)

            # ---- w_fft sums + broadcast to all partitions --------------
            # wft_sb[p, j*6+c] = wfT[j*128+p, c]; reduce over j then over
            # partitions (via ones-matmul, which also broadcasts), giving
            # bc2[m, c] = 2 * sum_k wfT[k, c] on every partition m.
            part6 = singles.tile([128, 6], F32)
            nc.vector.tensor_reduce(
                out=part6,
                in_=wft_sb.rearrange("p (j c) -> p c j", c=6),
                axis=AX.X,
                op=OP.add,
            )
            psum_bc = ps1.tile([128, 6], F32)
            nc.tensor.matmul(psum_bc, lhsT=ones, rhs=part6, start=True, stop=True)
            # bc2 on DVE (not ACT): the ACT instruction struct supports only
            # a single sem wait, so every ACT op must have all its
            # cross-engine producers on one proc (DVE).
            bc2 = singles.tile([128, 6], F32)
            nc.vector.tensor_scalar_mul(bc2, psum_bc, 2.0)

            # ---- 3-layer complex-tanh chain on [128, 16] ---------------
            x0 = chain.tile([128, 16], F32, tag="x0")
            nc.vector.tensor_copy(x0, xin_sb)
            x = x0
            for i in range(3):
                sr2 = bc2[:, 2 * i : 2 * i + 1]
                si2 = bc2[:, 2 * i + 1 : 2 * i + 2]
                T = chain.tile([128, 16], F32, tag="T")
                nc.scalar.activation(T, x, AF.Tanh, scale=sr2)
                b2 = chain.tile([128, 16], F32, tag="b2")
                nc.vector.tensor_scalar_mul(b2, x, si2)
                # cos(z): clamp z to ±30 (beyond that sech(2a) < 1e-23 kills
                # the cos term since |a| >= 0.93|z|), then start from
                # cos(z/32) = sin(z/32 + π/2) — argument within the Sin
                # table's [-π, π] domain — and apply the double-angle formula
                # cos(2θ) = 2cos²θ − 1 five times.
                b2c = chain.tile([128, 16], F32, tag="b2c")
                nc.vector.tensor_scalar(b2c, b2, -30.0, 30.0, OP.max, OP.min)
                C = chain.tile([128, 16], F32, tag="C")
                nc.scalar.activation(C, b2c, AF.Sin, bias=pihalf, scale=1.0 / 32.0)
                for _ in range(5):
                    Csq = chain.tile([128, 16], F32, tag="Csq")
                    nc.vector.tensor_mul(Csq, C, C)
                    C = chain.tile([128, 16], F32, tag="C")
                    nc.vector.tensor_scalar(C, Csq, 2.0, 1.0, OP.mult, OP.subtract)
                T2 = chain.tile([128, 16], F32, tag="T2")
                nc.vector.tensor_mul(T2, T, T)
                S = chain.tile([128, 16], F32, tag="S")
                nc.scalar.activation(S, T2, AF.Sqrt, bias=1.0, scale=-1.0)
                CS = chain.tile([128, 16], F32, tag="CS")
                nc.vector.tensor_mul(CS, C, S)
                D = chain.tile([128, 16], F32, tag="D")
                nc.vector.tensor_scalar_add(D, CS, 1.0)
                Rv = chain.tile([128, 16], F32, tag="Rv")
                nc.vector.reciprocal(Rv, D)
                xn = chain.tile([128, 16], F32, tag="xn")
                nc.vector.tensor_mul(xn, T, Rv)
                x = xn

            # The PE matmul LoadWeights struct also supports only one sem
            # wait. Real matmuls depend on both a weight-tile DMA and a
            # DVE/ACT-produced operand; that would be two waits. Fix: a
            # "warmup" matmul per weight tile whose only cross-engine dep is
            # that tile's DMA — after it, PE has observed the DMA lane tick
            # and the real matmuls need at most one wait.
            psum_dummy = ps1.tile([128, 1], F32)

            def warm(t):
                nc.tensor.matmul(
                    psum_dummy,
                    lhsT=t[:, 0:128],
                    rhs=ones[:, 0:1],
                    start=True,
                    stop=True,
                )

            # Same single-wait story on DVE: a tensor_tensor may depend on
            # PE (psum) + a DMA'd operand + a recycled-slot WAR — too many.
            # A 1-element DVE "touch" read of a freshly DMA'd tile makes DVE
            # observe that DMA lane's tick so the real op needs only the PE
            # wait.
            def touch(ap):
                tt = chain.tile([1, 1], F32, tag="touch")
                nc.vector.tensor_copy(tt, ap[0:1, 0:1])

            touch(l1b_sb)

            # lin1 weights stream through the same pool as lin2 tiles
            l1w_tiles = []
            for i in range(4):
                t = w2pool.tile([128, W2_NTILE], F32, tag="wtile")
                nc.sync.dma_start(
                    out=t, in_=l1wt[:, i * W2_NTILE : (i + 1) * W2_NTILE]
                )
                warm(t)
                l1w_tiles.append(t)

            # ---- lin1 shard: h[m,n] = tanh(sum_k l1wT[k, n*128+m]*x3[k] + b)
            psum_h = ps1.tile([128, 4], F32)
            for n in range(4):
                for kc in range(16):
                    lt = l1w_tiles[kc // 4]
                    base = (kc % 4) * SH
                    nc.tensor.matmul(
                        psum_h[:, n : n + 1],
                        lhsT=lt[:, base + n * 128 : base + (n + 1) * 128],
                        rhs=x[:, kc : kc + 1],
                        start=(kc == 0),
                        stop=(kc == 15),
                    )
            hb = singles.tile([128, 4], F32)
            nc.vector.tensor_add(hb, psum_h, l1b_sb)
            h = singles.tile([128, 4], F32)
            nc.scalar.activation(h, hb, AF.Tanh)

            # ---- broadcast h to all 128 partitions via a DRAM round trip
            # (write h[p, kc] to flat kc*128+p, read back with a stride-0
            # partition AP — the DMA-replicate pattern tile_groupnorm uses).
            nc.sync.dma_start(
                out=bass.AP(
                    tensor=h_dram[0:1, :].tensor,
                    offset=0,
                    ap=[[1, 128], [128, 4]],
                ),
                in_=h,
            )
            h_bc = singles.tile([128, SH], F32)
            nc.sync.dma_start(
                out=h_bc,
                in_=bass.AP(
                    tensor=h_dram[0:1, :].tensor,
                    offset=0,
                    ap=[[0, 128], [1, SH]],
                ),
            )
            touch(h_bc)

            # ---- lin2 is split across PE and DVE (fp32 rates are similar;
            # running both halves concurrently nearly halves the compute
            # span): PE covers outputs [0, NPE) in the transposed layout,
            # DVE covers [NPE, 16384) rows-on-partitions via mul+reduce.
            y_dve = singles.tile([128, NRB], F32)
            scr = singles.tile([128, SH], F32)

            def dve_tile(g):
                tv = w2vp.tile([128, 8 * SH], F32, tag="wv")
                nc.sync.dma_start(out=tv, in_=w2v[:, g * 8 * SH : (g + 1) * 8 * SH])
                touch(tv)
                return tv

            def dve_block(tv, g):
                for rr in range(8):
                    r = g * 8 + rr
                    nc.vector.tensor_mul(
                        scr, tv[:, rr * SH : (rr + 1) * SH], h_bc
                    )
                    nc.vector.tensor_reduce(
                        out=y_dve[:, r : r + 1], in_=scr, axis=AX.X, op=OP.add
                    )

            # PE part: stripes in the transposed layout (unchanged scheme)
            yslab = singles.tile([1, NPE], F32)
            N_DVE_TILES = NRB // 8
            dve_tiles = [dve_tile(0), dve_tile(1), dve_tile(2)]
            for j in range(PE_NJ):
                wt = []
                for kc in range(4):
                    t = w2pool.tile([128, W2_NTILE], F32, tag="wtile")
                    nc.sync.dma_start(
                        out=t,
                        in_=w2t[:, kc * NPE + j * W2_NTILE : kc * NPE + (j + 1) * W2_NTILE],
                    )
                    warm(t)
                    wt.append(t)
                bias_t = biasp.tile([1, W2_NTILE], F32, tag="bias")
                nc.sync.dma_start(
                    out=bias_t, in_=l2b[0:1, j * W2_NTILE : (j + 1) * W2_NTILE]
                )
                # PE-observe the bias DMA so the bias matmuls need no wait
                nc.tensor.matmul(
                    psum_dummy[0:1, 0:1],
                    lhsT=bias_t[0:1, 0:1],
                    rhs=ones[0:1, 0:1],
                    start=True,
                    stop=True,
                )
                for nb in range(W2_NB):
                    ps = psy.tile([1, 512], F32, tag="psy")
                    for kc in range(4):
                        nc.tensor.matmul(
                            ps,
                            lhsT=h[:, kc : kc + 1],
                            rhs=wt[kc][:, nb * 512 : (nb + 1) * 512],
                            start=(kc == 0),
                            stop=False,
                        )
                    nc.tensor.matmul(
                        ps,
                        lhsT=ones[0:1, 0:1],
                        rhs=bias_t[0:1, nb * 512 : (nb + 1) * 512],
                        start=False,
                        stop=True,
                    )
                    n0 = j * W2_NTILE + nb * 512
                    # copy on the (otherwise idle) scalar engine: keeps the
                    # DVE free for its half of lin2
                    nc.scalar.copy(yslab[0:1, n0 : n0 + 512], ps)
                # interleave: after each PE stripe, emit one DVE tile's DMA
                # + compute so both engines stream concurrently
                if j + 3 < N_DVE_TILES:
                    dve_tiles.append(dve_tile(j + 3))
                dve_block(dve_tiles[j], j)
            nc.sync.dma_start(out=yout[0:1, :], in_=yslab)
            for g in range(PE_NJ, N_DVE_TILES):
                while len(dve_tiles) < min(N_DVE_TILES, g + 4):
                    dve_tiles.append(dve_tile(len(dve_tiles)))
                dve_block(dve_tiles[g], g)
            nc.vector.tensor_add(y_dve, y_dve, b2v_sb)
            nc.sync.dma_start(out=yout2[:, :], in_=y_dve)

    fixed = _legalize_wait_counts(nc.to_json_bytes())
    nc.to_json_bytes = lambda: fixed
    return nc


def make_in_maps(inputs):
    """Host-side shard/relayout of the full inputs into per-core arrays."""
    model_p = np.asarray(inputs["model_p"])
    v = np.ascontiguousarray(model_p[:, 0]).astype(np.float32)          # (2048,)
    xin = np.ascontiguousarray(v.reshape(16, 128).T)                    # (128,16)

    wfs = []
    for k in ("w_fft_0", "w_fft_1", "w_fft_2"):
        w = np.asarray(inputs[k]).reshape(MODES)
        wfs.append(np.ascontiguousarray(w.real).astype(np.float32))
        wfs.append(np.ascontiguousarray(w.imag).astype(np.float32))
    wfT = np.stack(wfs, axis=1)                                         # (4096, 6)
    wft = np.ascontiguousarray(
        wfT.reshape(32, 128, 6).transpose(1, 0, 2).reshape(128, 192)
    )

    lin1_w = np.asarray(inputs["lin1_w"], dtype=np.float32)             # (4096, 2048)
    lin1_b = np.asarray(inputs["lin1_b"], dtype=np.float32)             # (4096,)
    lin2_w = np.asarray(inputs["lin2_w"], dtype=np.float32)             # (16384, 4096)
    lin2_b = np.asarray(inputs["lin2_b"], dtype=np.float32)             # (16384,)

    in_maps = []
    for c in range(N_CORES):
        l1wT = lin1_w[c * SH : (c + 1) * SH, :].T                       # (2048, 512)
        l1wt = np.ascontiguousarray(
            l1wT.reshape(16, 128, SH).transpose(1, 0, 2).reshape(128, 16 * SH)
        )
        l1bc = np.ascontiguousarray(
            lin1_b[c * SH : (c + 1) * SH].reshape(4, 128).T
        )                                                               # (128, 4)
        w2T = lin2_w[:NPE, c * SH : (c + 1) * SH].T                     # (512, 6144)
        w2tc = np.ascontiguousarray(
            w2T.reshape(4, 128, NPE).transpose(1, 0, 2).reshape(128, 4 * NPE)
        )
        # DVE half: rows-on-partitions, w2vc[p, r*512+k] = W2c[NPE+r*128+p, k]
        w2c_v = lin2_w[NPE:, c * SH : (c + 1) * SH]                     # (10240, 512)
        w2vc = np.ascontiguousarray(
            w2c_v.reshape(NRB, 128, SH).transpose(1, 0, 2).reshape(128, NRB * SH)
        )
        if c == 0:
            b2vc = np.ascontiguousarray(lin2_b[NPE:].reshape(NRB, 128).T)
        else:
            b2vc = np.zeros((128, NRB), np.float32)
        if c == 0:
            l2bc = np.ascontiguousarray(lin2_b.reshape(1, M_OUT))
        else:
            l2bc = np.zeros((1, M_OUT), np.float32)
        in_maps.append(
            {
                "xin": xin,
                "wft": wft,
                "l1wt": l1wt,
                "l1b": l1bc,
                "w2t": w2tc,
                "w2v": w2vc,
                "b2v": b2vc,
                "l2b": l2bc,
            }
        )
    return in_maps


_CACHE = {}


def kernel(**inputs) -> np.ndarray:
    from concourse.bass_utils import run_bass_kernel_spmd

    if "nc" not in _CACHE:
        _CACHE["nc"] = build_nc()
    nc = _CACHE["nc"]

    in_maps = make_in_maps(inputs)
    res = run_bass_kernel_spmd(nc, in_maps, list(range(N_CORES)))
    y = np.zeros(M_OUT, np.float64)
    for c in range(N_CORES):
        y[:NPE] += res.results[c]["yout"].reshape(NPE).astype(np.float64)
        # yout2[p, r] = y_partial[NPE + r*128 + p]
        y[NPE:] += res.results[c]["yout2"].T.reshape(M_OUT - NPE).astype(np.float64)
    return y.astype(np.float32).reshape(1, NO_LAYERS, APL)
